# revision 58
# baseline (speedup 1.0000x reference)
"""ALBEF concept-text contrastive loss on 8 TRN2 NeuronCores (v5).

Design (per core r):
  * Text columns PACKED: each batch row j keeps only its valid words,
    padded to lam_j = 8*ceil(nw_j/8) (pad = copies of j's first valid
    word, mask weight 0). Bucket counts promoted to multiples of 8 so
    every core's 32-row text block has the IDENTICAL run structure
    (SPMD-safe); text rows are assigned to cores by bucket, decoupled
    from the concept sharding. TPW ~ 850 packed cols vs 1280 dense.
  * S computed as fp8 DoubleRow matmuls (K=256 in one pass), chunks of
    <=512 cols; colmax (term_col) via DVE segmented reduce per
    equal-lam run; term_row max_q via sharp LSE (ACT exp, gpsimd
    accumulate-DMA pair sums, DVE fold, one eq matmul per chunk).
  * OWN-BLOCK FIRST: core computes S for its own text block from local
    wfl8 into staging columns (g=8) while the runtime CC-init barrier
    (~43us) and first AllGather complete; the AG duplicate of the own
    block is excluded from row sums via a per-core cvalid mask and the
    host-side combine.
  * Loss: per-core [1, BT+1] partials (column exp sums + row-lse/diag
    scalar) are summed on the HOST (the unshard step) - no AllReduce.
"""

import ml_dtypes
import numpy as np

import concourse.bass as bass
import concourse.bacc as bacc
import concourse.mybir as mybir
import concourse.tile as tile
from concourse.bass_utils import run_bass_kernel_spmd

F32 = mybir.dt.float32
BF16 = mybir.dt.bfloat16
FP8 = mybir.dt.float8e4
AX = mybir.AxisListType
ALU = mybir.AluOpType
ACTF = mybir.ActivationFunctionType
PM = mybir.MatmulPerfMode

B, Q, L, VW, TW, D = 256, 32, 40, 768, 768, 256
NCORES = 8
BL = B // NCORES            # 32 local batch rows
IQ = BL * Q                 # 1024 local (q,i) columns, q-major
KC = VW // 128              # 6 contraction chunks for projection
NM = IQ // 128              # 8 m-chunks of concept rows
BT = B + BL                 # 288 sim columns (8 AG blocks + own staging)

FSC = 16.0                  # fp8 feature scale (S scaled by FSC^2=256)
BHAT = 0.375                # exp scale on scaled S (beta_orig = 96)
SHIFT = 12.0                # exp arg = BHAT*S_tilde - SHIFT
CMAX = 512                  # max matmul N / PSUM bank cols

_CACHE = {}


def _layout(text_mask):
    """Packed-column layout from the actual mask (graph-shaping)."""
    m = text_mask.astype(np.int32).copy()
    m[:, 0] = 0
    sep = (L - 1) - np.argmax(m[:, ::-1] > 0, axis=1)
    m[np.arange(B), sep] = 0
    nw = m.sum(axis=1)
    assert nw.min() >= 1
    lam = (8 * np.ceil(nw / 8.0)).astype(np.int64)
    # promote so each bucket count is a multiple of 8 (identical
    # per-core run structure); last bucket is then automatically ok
    for lv in sorted(set(lam.tolist()))[:-1]:
        js = np.where(lam == lv)[0]
        extra = len(js) % 8
        if extra:
            lam[js[np.argsort(-nw[js])[:extra]]] += 8
    vals = sorted(set(lam.tolist()), reverse=True)
    assert all((lam == lv).sum() % 8 == 0 for lv in vals)
    # deal each bucket round-robin to cores; order within core: lam desc
    order = [[] for _ in range(NCORES)]
    for lv in vals:
        js = np.where(lam == lv)[0]
        for c in range(NCORES):
            order[c].extend(js[c::NCORES].tolist())
    runs = [(lv, int((lam == lv).sum()) // 8) for lv in vals]
    tpw = int(sum(lv * nj for lv, nj in runs))
    # chunks: cut runs into <=CMAX-wide pieces, preferring run
    # boundaries (fewer reduce segments) once a chunk is >=320 wide
    chunks = []
    cur, cw, jidx = [], 0, 0
    for lv, nj in runs:
        if cw >= 320 and cw + lv * nj > CMAX:
            chunks.append((cw, cur))
            cur, cw = [], 0
        rem = nj
        while rem:
            k = min(rem, (CMAX - cw) // lv)
            if k == 0:
                chunks.append((cw, cur))
                cur, cw = [], 0
                continue
            cur.append((cw, k, lv, jidx))
            cw += k * lv
            jidx += k
            rem -= k
    if cur:
        chunks.append((cw, cur))
    # (coff, w, parts) with parts = [(off_in_chunk, nj, lam, jbase)]
    out, off = [], 0
    for w, parts in chunks:
        out.append((off, w, parts))
        off += w
    assert off == tpw
    return {"m": m, "nw": nw, "lam": lam, "order": order, "runs": runs,
            "tpw": tpw, "chunks": out}


def _build(lay):
    nc = _build_graph(lay)
    nc.compile()
    return nc


def _build_graph(lay):
    import os
    TPW = lay["tpw"]
    CH = lay["chunks"]
    MT = (TPW + 127) // 128          # text projection m-chunks
    GW0 = CH[0][1]                   # first AG group = first chunk cols
    G0M = (GW0 + 127) // 128         # text m-chunks completing group 0
    AGR = [(0, GW0), (GW0, TPW - GW0)]

    nc = bacc.Bacc("TRN2", target_bir_lowering=False, debug=False,
                   num_devices=NCORES)

    concept_t = nc.dram_tensor("concept_t", [VW, IQ], FP8, kind="ExternalInput")
    text_t = nc.dram_tensor("text_t", [TW, TPW], FP8, kind="ExternalInput")
    wc = nc.dram_tensor("wc", [VW, D], FP8, kind="ExternalInput")
    ww = nc.dram_tensor("ww", [TW, D], FP8, kind="ExternalInput")
    brows = nc.dram_tensor("brows", [1, 2 * D], BF16, kind="ExternalInput")
    ones_row = nc.dram_tensor("ones_row", [1, 128], BF16, kind="ExternalInput")
    ident_bf = nc.dram_tensor("ident_bf", [128, 128], BF16, kind="ExternalInput")
    eqmat = nc.dram_tensor("eqmat", [128, BL], BF16, kind="ExternalInput")
    eones = nc.dram_tensor("eones", [128, BL], BF16, kind="ExternalInput")
    maskw = nc.dram_tensor("maskw", [1, 9 * TPW], BF16, kind="ExternalInput")
    dmask = nc.dram_tensor("dmask", [BL, BT], F32, kind="ExternalInput")
    cvalid = nc.dram_tensor("cvalid", [BL, BT], BF16, kind="ExternalInput")
    ones32 = nc.dram_tensor("ones32", [BL, 1], F32, kind="ExternalInput")

    pout = nc.dram_tensor("pout", [1, BT + 1], F32, kind="ExternalOutput")
    dbg = None
    if os.environ.get("KDBG"):
        dbg = nc.dram_tensor("dbg", [BL, BT], F32, kind="ExternalOutput")

    # collective buffers (outputs Shared for fast HBM-HBM paths)
    ag_in = [nc.dram_tensor(f"ag_in{g}", [2 * 128, w], FP8, kind="Internal")
             for g, (_, w) in enumerate(AGR)]
    ag_out = [nc.dram_tensor(f"ag_out{g}", [NCORES * 2 * 128, w], FP8,
                             kind="Internal", addr_space="Shared")
              for g, (_, w) in enumerate(AGR)]

    with tile.TileContext(nc) as tc:
        with (
            tc.tile_pool(name="cst", bufs=1) as cst,
            tc.tile_pool(name="feat", bufs=1) as feat,
        ):
            # ---- persistent SBUF tiles ----
            cf8 = feat.tile([128, 2, IQ], FP8, tag="cf8")
            wfl8 = feat.tile([128, 2, TPW], FP8, tag="wfl8")
            wf8g = feat.tile([128, 2, NCORES * TPW], FP8, tag="wf8g")
            cm_all = feat.tile([128, NM * BT], BF16, tag="cm_all")
            trow_sb = feat.tile([BL, BT], F32, tag="trow_sb")
            sim_sb = feat.tile([BL, BT], F32, tag="sim_sb")
            qsf = feat.tile([BL, 9 * TPW], F32, tag="qsf")

            ident_sb = cst.tile([128, 128], BF16, tag="ident_sb")
            eq_sb = cst.tile([128, BL], BF16, tag="eq_sb")
            eones_sb = cst.tile([128, BL], BF16, tag="eones_sb")
            maskw_sb = cst.tile([BL, 9 * TPW], BF16, tag="maskw_sb")
            dmask_sb = cst.tile([BL, BT], F32, tag="dmask_sb")
            cvalid_sb = cst.tile([BL, BT], BF16, tag="cvalid_sb")
            ones32_sb = cst.tile([BL, 1], F32, tag="ones32_sb")
            ones32b_sb = cst.tile([BL, 1], BF16, tag="ones32b_sb")
            onesr_sb = cst.tile([1, 128], BF16, tag="onesr_sb")
            brow_sb = cst.tile([1, 2 * D], BF16, tag="brow_sb")
            shift_sb = cst.tile([128, 1], F32, tag="shift_sb")

            nc.sync.dma_start(ident_sb[:], ident_bf[:])
            nc.sync.dma_start(eq_sb[:], eqmat[:])
            nc.sync.dma_start(eones_sb[:], eones[:])
            nc.sync.dma_start(dmask_sb[:], dmask[:])
            nc.sync.dma_start(cvalid_sb[:], cvalid[:])
            nc.sync.dma_start(ones32_sb[:], ones32[:])
            nc.vector.memset(ones32b_sb[:], 1.0)
            nc.vector.memset(shift_sb[:], -SHIFT)
            nc.sync.dma_start(onesr_sb[:], ones_row[:])
            nc.sync.dma_start(brow_sb[:], brows[:])
            # load mask row once, replicate to 32 partitions by doubling
            nc.sync.dma_start(maskw_sb[0:1, :], maskw[:])
            for k in (1, 2, 4, 8, 16):
                nc.sync.dma_start(maskw_sb[k:2 * k, :], maskw_sb[0:k, :])
            # preload Square/Sqrt/Exp/Ln tables while DMAs run
            warm = cst.tile([1, 4], F32, tag="warm")
            nc.scalar.activation(warm[0:1, 3:4], ones32_sb[0:1, :], ACTF.Square)
            nc.scalar.activation(warm[0:1, 0:1], ones32_sb[0:1, :], ACTF.Sqrt)
            nc.scalar.activation(warm[0:1, 1:2], ones32_sb[0:1, :], ACTF.Exp)
            nc.scalar.activation(warm[0:1, 2:3], ones32_sb[0:1, :], ACTF.Ln)

            def issue_ag(g):
                off, w = AGR[g]
                nc.sync.dma_start(
                    ag_in[g][:].rearrange("(k p) w -> p k w", p=128),
                    wfl8[:, :, off:off + w])
                nc.gpsimd.collective_compute(
                    "AllGather", ALU.bypass,
                    ins=[ag_in[g][:]], outs=[ag_out[g][:]],
                    replica_groups=[list(range(NCORES))])
                for rr in range(NCORES):
                    for k in range(2):
                        nc.sync.dma_start(
                            wf8g[:, k, rr * TPW + off:rr * TPW + off + w],
                            ag_out[g][:].rearrange(
                                "(rr k p) w -> p k rr w",
                                k=2, p=128)[:, k, rr, :])

            # ---- stage 1: projections + l2norm -> fp8 transposed feats ----
            with (
                tc.tile_pool(name="pin", bufs=1) as pin,
                tc.tile_pool(name="ps2", bufs=3, space="PSUM") as ps2,
                tc.tile_pool(name="pst", bufs=4, space="PSUM") as pst,
                tc.tile_pool(name="wk2", bufs=3) as wk2,
            ):
                TPWP = ((TPW + 127) // 128) * 128
                tin = pin.tile([128, KC * TPWP], FP8, tag="tin")
                cin = pin.tile([128, KC * IQ], FP8, tag="cin")
                wcs = pin.tile([128, KC * D], FP8, tag="wcs")
                wws = pin.tile([128, KC * D], FP8, tag="wws")
                nc.sync.dma_start(
                    tin[:].rearrange("p (k j) -> p k j", j=TPWP)[:, :, 0:TPW],
                    text_t[:].rearrange("(k p) j -> p k j", p=128))
                nc.sync.dma_start(
                    wws[:].rearrange("p (k d) -> p k d", d=D),
                    ww[:].rearrange("(k p) d -> p k d", p=128))
                nc.sync.dma_start(
                    cin[:].rearrange("p (k j) -> p k j", j=IQ),
                    concept_t[:].rearrange("(k p) j -> p k j", p=128))
                nc.sync.dma_start(
                    wcs[:].rearrange("p (k d) -> p k d", d=D),
                    wc[:].rearrange("(k p) d -> p k d", p=128))

                def project(src, width, vwidth, w_sb, brow_ix, dst8, ms,
                            after_m=None):
                    srcv = src[:].rearrange("p (k j) -> p k j", j=vwidth)
                    wv = w_sb[:].rearrange("p (k d) -> p k d", d=D)
                    for m in ms:
                        mw = min(128, width - m * 128)
                        pp = ps2.tile([128, D], F32, tag="pp")
                        if mw >= 32:
                            for k in range(KC // 2):
                                nc.tensor.matmul(
                                    pp[0:mw, :],
                                    lhsT=srcv[:, 2 * k:2 * k + 2,
                                              m * 128:m * 128 + mw],
                                    rhs=wv[:, 2 * k:2 * k + 2, :],
                                    start=(k == 0), stop=False,
                                    perf_mode=PM.DoubleRow)
                        else:
                            for k in range(KC):
                                nc.tensor.matmul(
                                    pp[0:mw, :],
                                    lhsT=srcv[:, k:k + 1,
                                              m * 128:m * 128 + mw],
                                    rhs=wv[:, k:k + 1, :],
                                    start=(k == 0), stop=False)
                        nc.tensor.matmul(
                            pp[0:mw, :], lhsT=onesr_sb[0:1, 0:mw],
                            rhs=brow_sb[0:1, brow_ix * D:(brow_ix + 1) * D],
                            start=False, stop=True, skip_group_check=True)
                        sq = wk2.tile([128, D], BF16, tag="sq")
                        ss = wk2.tile([128, 1], F32, tag="ss")
                        nc.scalar.activation(sq[0:mw, :], pp[0:mw, :],
                                             ACTF.Square, accum_out=ss[0:mw, :])
                        rcp = wk2.tile([128, 1], F32, tag="rcp")
                        nc.vector.reciprocal(rcp[0:mw, :], ss[0:mw, :])
                        rn = wk2.tile([128, 1], F32, tag="rn")
                        # rn = FSC / sqrt(ss)
                        nc.scalar.activation(rn[0:mw, :], rcp[0:mw, :],
                                             ACTF.Sqrt, scale=FSC * FSC)
                        nrm = wk2.tile([128, D], BF16, tag="nrm")
                        nc.vector.tensor_scalar_mul(nrm[0:mw, :], pp[0:mw, :],
                                                    rn[0:mw, :])
                        for kk in range(2):
                            ptr = pst.tile([128, 128], BF16, tag="ptr")
                            nc.tensor.transpose(
                                ptr[:, 0:mw],
                                nrm[0:mw, kk * 128:(kk + 1) * 128],
                                ident_sb[0:mw, 0:mw])
                            nc.vector.tensor_scalar_mul(
                                dst8[:, kk, m * 128:m * 128 + mw],
                                ptr[:, 0:mw], 1.0)
                        if after_m is not None and m in after_m:
                            after_m[m]()

                project(tin, TPW, TPWP, wws, 1, wfl8, ms=range(0, G0M),
                        after_m={G0M - 1: lambda: issue_ag(0)})
                project(cin, IQ, IQ, wcs, 0, cf8, ms=range(NM))
                project(tin, TPW, TPWP, wws, 1, wfl8, ms=range(G0M, MT),
                        after_m={MT - 1: lambda: issue_ag(1)})

            # ---- stage 2: main pass (own block first, then AG blocks) ----
            with (
                tc.tile_pool(name="ptc", bufs=1, space="PSUM") as ptc,
                tc.tile_pool(name="psa", bufs=3, space="PSUM") as psa,
                tc.tile_pool(name="psq", bufs=1, space="PSUM") as psq,
                tc.tile_pool(name="wke", bufs=8) as wke,
                tc.tile_pool(name="wkl", bufs=3) as wkl,
            ):
                term_col = ptc.tile([BL, BT], F32, tag="term_col")

                def do_chunk(ci, g, src, scol):
                    # g: 0..7 = AG rank block, 8 = own staging block
                    coff, w, parts = CH[ci]
                    qcol = g * TPW + coff
                    exs = []
                    for pr in range(4):
                        pa = psa.tile([128, 1024], F32, tag="pa")
                        for h in range(2):
                            m = 2 * pr + h
                            nc.tensor.matmul(
                                pa[:, h * 512:h * 512 + w],
                                lhsT=cf8[:, :, m * 128:(m + 1) * 128],
                                rhs=src[:, :, scol:scol + w],
                                start=True, stop=True,
                                perf_mode=PM.DoubleRow)
                        # segmented max over l per equal-lam run
                        for poff, nj, lv, jb in parts:
                            nc.vector.tensor_reduce(
                                cm_all[:].rearrange("p (m j) -> p m j",
                                                    j=BT)[
                                    :, 2 * pr:2 * pr + 2,
                                    g * BL + jb:g * BL + jb + nj],
                                pa[:].rearrange("p (h x) -> p h x", x=512)[
                                    :, :, poff:poff + nj * lv].rearrange(
                                    "p h (j l) -> p h j l", l=lv),
                                axis=AX.X, op=ALU.max)
                        ex = wke.tile([128, 2, 512], BF16, tag="ex")
                        nc.scalar.activation(
                            ex[:, :, 0:w],
                            pa[:].rearrange("p (h x) -> p h x",
                                            x=512)[:, :, 0:w],
                            ACTF.Exp, scale=BHAT, bias=shift_sb[:])
                        exs.append(ex)
                    # sum the 4 exp tiles: 2 accum-DMAs + 1 DVE add,
                    # then fold the two 512-halves on DVE
                    nc.gpsimd.dma_start(exs[0][:, :, 0:w],
                                        exs[1][:, :, 0:w],
                                        accum_op=ALU.add)
                    nc.gpsimd.dma_start(exs[2][:, :, 0:w],
                                        exs[3][:, :, 0:w],
                                        accum_op=ALU.add)
                    qs = psq.tile([BL, 512], F32, tag="qs")
                    for t in range(2):
                        for h in range(2):
                            nc.tensor.matmul(qs[0:BL, 0:w], lhsT=eq_sb[:],
                                             rhs=exs[2 * t][:, h, 0:w],
                                             start=(t == 0 and h == 0),
                                             stop=(t == 1 and h == 1))
                    nc.scalar.copy(qsf[:, qcol:qcol + w], qs[0:BL, 0:w])

                def ln_own(ci):
                    # own block: Ln / mask / l-sum into staging cols g=8
                    coff, w, parts = CH[ci]
                    qcol = 8 * TPW + coff
                    lns = wkl.tile([BL, w], BF16, tag="lnso")
                    nc.scalar.activation(lns[:], qsf[:, qcol:qcol + w],
                                         ACTF.Ln)
                    lnw = wkl.tile([BL, w], BF16, tag="lnwo")
                    nc.vector.tensor_tensor(lnw[:], lns[:],
                                            maskw_sb[:, qcol:qcol + w],
                                            op=ALU.mult)
                    for poff, nj, lv, jb in parts:
                        nc.vector.tensor_reduce(
                            trow_sb[:, 8 * BL + jb:8 * BL + jb + nj],
                            lnw[:, poff:poff + nj * lv].rearrange(
                                "p (j l) -> p j l", l=lv),
                            axis=AX.X, op=ALU.add)

                def ln_batch(ci, r0=0, r1=NCORES):
                    # AG blocks: batched Ln / mask-mult over rank range
                    # [r0, r1) (3D APs, stride TPW)
                    coff, w, parts = CH[ci]
                    nr = r1 - r0
                    qv = qsf[:, 0:NCORES * TPW].rearrange(
                        "p (rr x) -> p rr x",
                        x=TPW)[:, r0:r1, coff:coff + w]
                    lns = wkl.tile([BL, NCORES, 512], BF16, tag="lns")
                    nc.scalar.activation(lns[0:BL, 0:nr, 0:w], qv, ACTF.Ln)
                    mv = maskw_sb[:, 0:NCORES * TPW].rearrange(
                        "p (rr x) -> p rr x",
                        x=TPW)[:, r0:r1, coff:coff + w]
                    lnw = wkl.tile([BL, NCORES, 512], BF16, tag="lnw")
                    nc.vector.tensor_tensor(lnw[0:BL, 0:nr, 0:w],
                                            lns[0:BL, 0:nr, 0:w], mv,
                                            op=ALU.mult)
                    for poff, nj, lv, jb in parts:
                        nc.vector.tensor_reduce(
                            trow_sb[:].rearrange("p (g j) -> p g j",
                                                 j=BL)[:, r0:r1,
                                                       jb:jb + nj],
                            lnw[0:BL, 0:nr,
                                poff:poff + nj * lv].rearrange(
                                "p rr (j l) -> p rr j l", l=lv),
                            axis=AX.X, op=ALU.add)

                # own block from local features (fills the CC-init stall)
                for ci in range(len(CH)):
                    do_chunk(ci, 8, wfl8, CH[ci][0])
                    ln_own(ci)
                def term_col_ci(ci):
                    # fold this ci's colmax cols over q now (j cols of a
                    # chunk are contiguous within every block)
                    j0 = CH[ci][2][0][3]
                    j1 = CH[ci][2][-1][3] + CH[ci][2][-1][1]
                    for m in range(NM):
                        nc.tensor.matmul(
                            term_col[:].rearrange("p (g j) -> p g j",
                                                  j=BL)[:, :, j0:j1],
                            lhsT=eones_sb[:],
                            rhs=cm_all[:, m * BT:(m + 1) * BT].rearrange(
                                "p (g j) -> p g j", j=BL)[:, :, j0:j1],
                            start=(m == 0), stop=(m == NM - 1))

                # gathered blocks; defer each ci's Ln/term_col epilogue
                # into the next ci's compute to avoid boundary stalls
                NCH = len(CH)
                for ci in range(NCH):
                    for rr in range(NCORES):
                        do_chunk(ci, rr, wf8g, rr * TPW + CH[ci][0])
                        if ci > 0 and rr == 2:
                            ln_batch(ci - 1, 0, 4)
                        if ci > 0 and rr == 4:
                            ln_batch(ci - 1, 4, NCORES)
                        if ci > 0 and rr == 6:
                            term_col_ci(ci - 1)
                        # last ci: drain ranks 0-5 early so only 6-7
                        # remain on the serial tail
                        if ci == NCH - 1 and rr == 6:
                            ln_batch(ci, 0, 6)
                ln_batch(NCH - 1, 6, NCORES)
                term_col_ci(NCH - 1)

                # ---- stage 3: sim ----
                nc.vector.tensor_tensor(sim_sb[:], term_col[:], trow_sb[:],
                                        op=ALU.add)
                if dbg is not None:
                    nc.sync.dma_start(dbg[:], sim_sb[:])

            # ---- loss partials; cross-core combine happens on host ----
            with (
                tc.tile_pool(name="ps7", bufs=1, space="PSUM") as ps7,
                tc.tile_pool(name="wk7", bufs=1) as wk7,
            ):
                # sim is O(+-30): exp needs no max shift in f32/bf16
                escr = wk7.tile([BL, BT], BF16, tag="escr")
                nc.scalar.activation(escr[:], sim_sb[:], ACTF.Exp)
                escv = wk7.tile([BL, BT], BF16, tag="escv")
                sume = wk7.tile([BL, 1], F32, tag="sume")
                nc.vector.scalar_tensor_tensor(
                    escv[:], escr[:], 1.0, cvalid_sb[:],
                    op0=ALU.mult, op1=ALU.mult, accum_out=sume[:])
                lg = wk7.tile([BL, 1], F32, tag="lg")
                nc.scalar.activation(lg[:], sume[:], ACTF.Ln)
                dscr = wk7.tile([BL, BT], F32, tag="dscr")
                dg = wk7.tile([BL, 1], F32, tag="dg")
                nc.vector.scalar_tensor_tensor(
                    dscr[:], sim_sb[:], 1.0, dmask_sb[:],
                    op0=ALU.mult, op1=ALU.mult, accum_out=dg[:])
                # v = 2*dg - lse_row  [BL,1]
                v1 = wk7.tile([BL, 1], F32, tag="v1")
                nc.vector.scalar_tensor_tensor(
                    v1[:], dg[:], 2.0, lg[:],
                    op0=ALU.mult, op1=ALU.subtract)
                csum = ps7.tile([1, BT], F32, tag="csum")
                nc.tensor.matmul(csum[:], lhsT=ones32b_sb[:],
                                 rhs=escv[:], start=True, stop=True)
                ssum = ps7.tile([1, 1], F32, tag="ssum")
                nc.tensor.matmul(ssum[:], lhsT=ones32_sb[:], rhs=v1[:],
                                 start=True, stop=True)
                arv = wk7.tile([1, BT + 1], F32, tag="arv")
                nc.scalar.copy(arv[0:1, 0:BT], csum[0:1, :])
                nc.scalar.copy(arv[0:1, BT:BT + 1], ssum[0:1, :])
                nc.sync.dma_start(pout[:], arv[:])

    return nc


def _host_prep(inputs):
    concept_feat = np.ascontiguousarray(np.asarray(inputs["concept_feat"],
                                                   dtype=np.float32))
    text_embeds = np.asarray(inputs["text_embeds"], dtype=np.float32)
    text_mask = np.asarray(inputs["text_mask"]).astype(np.int32)
    Wc = np.ascontiguousarray(np.asarray(inputs["Wc"], dtype=np.float32))
    bc = np.asarray(inputs["bc"], dtype=np.float32)
    Ww = np.ascontiguousarray(np.asarray(inputs["Ww"], dtype=np.float32))
    bw = np.asarray(inputs["bw"], dtype=np.float32)
    temp = float(np.asarray(inputs["temp_cpt"]))

    lay = _layout(text_mask)
    m, nw, lam = lay["m"], lay["nw"], lay["lam"]
    order, TPW = lay["order"], lay["tpw"]

    # Eq: partition p=(qo,i) -> i one-hot; eones adds 1/(Q*temp*FSC^2)
    eq = np.zeros((128, BL), dtype=np.float32)
    eo = np.zeros((128, BL), dtype=np.float32)
    for p in range(128):
        eq[p, p % BL] = 1.0
        eo[p, p % BL] = 1.0 / (Q * temp * FSC * FSC)

    # packed per-core text blocks + global mask-weight row
    wscale = 1.0 / (temp * FSC * FSC * BHAT)
    blocks, wrow_blocks, pos_of = [], [], {}
    for r in range(NCORES):
        cols = np.empty((TPW, TW), dtype=np.float32)
        wrow = np.zeros((TPW,), dtype=np.float32)
        off = 0
        for pos, j in enumerate(order[r]):
            lv, nv = int(lam[j]), int(nw[j])
            pos_of[j] = (r, pos)
            valid = np.where(m[j] > 0)[0]
            cols[off:off + nv] = text_embeds[j, valid]
            cols[off + nv:off + lv] = text_embeds[j, valid[0]]
            wrow[off:off + nv] = wscale / nv
            off += lv
        assert off == TPW
        blocks.append(cols)
        wrow_blocks.append(wrow)
    maskw_g = np.concatenate(wrow_blocks)

    ident = np.eye(128, dtype=np.float32)
    ones_row = np.ones((1, 128), dtype=np.float32)
    ones32 = np.ones((BL, 1), dtype=np.float32)
    brows = np.concatenate([bc, bw])[None, :]

    # weights/bias fp8, x64 into e4m3 normal range; l2norm removes the
    # common scale so no unscaling is needed downstream
    WS = 64.0
    shared = {
        "wc": (Wc * WS).astype(ml_dtypes.float8_e4m3),
        "ww": (Ww * WS).astype(ml_dtypes.float8_e4m3),
        "brows": (brows * WS).astype(ml_dtypes.bfloat16),
        "ones_row": ones_row.astype(ml_dtypes.bfloat16),
        "ident_bf": ident.astype(ml_dtypes.bfloat16),
        "eqmat": eq.astype(ml_dtypes.bfloat16),
        "eones": eo.astype(ml_dtypes.bfloat16),
        "ones32": ones32,
    }
    in_maps = []
    for r in range(NCORES):
        im = dict(shared)
        im["concept_t"] = np.ascontiguousarray(
            concept_feat[r * BL:(r + 1) * BL].transpose(1, 0, 2)
            .reshape(IQ, VW).T).astype(ml_dtypes.float8_e4m3)
        im["text_t"] = np.ascontiguousarray(
            blocks[r].T).astype(ml_dtypes.float8_e4m3)
        im["maskw"] = np.concatenate(
            [maskw_g, wrow_blocks[r]])[None, :].astype(ml_dtypes.bfloat16)
        dmask_np = np.zeros((BL, BT), dtype=np.float32)
        cval = np.ones((BL, BT), dtype=np.float32)
        cval[:, r * BL:(r + 1) * BL] = 0.0
        for i in range(BL):
            tr, pos = pos_of[r * BL + i]
            col = 8 * BL + pos if tr == r else tr * BL + pos
            dmask_np[i, col] = 1.0
        im["dmask"] = dmask_np
        im["cvalid"] = cval.astype(ml_dtypes.bfloat16)
        in_maps.append(im)
    return in_maps, lay


def finalize(res, lay):
    # host-side unshard: sum the per-core [1, BT+1] partials; own-block
    # staging cols (g=8) fill the cvalid-zeroed own-rank AG slots
    parts = np.stack([np.asarray(res.results[r]["pout"][0],
                                 dtype=np.float64) for r in range(NCORES)])
    tot = parts[:, 0:B].sum(axis=0)
    for r in range(NCORES):
        tot[r * BL:(r + 1) * BL] += parts[r, B:BT]
    v = parts[:, BT].sum()
    fin = np.log(tot).sum() - v
    return np.float32(fin / (2 * B))


def _sig(lay):
    return (lay["tpw"], tuple(lay["runs"]),
            tuple((c, w, tuple(p)) for c, w, p in lay["chunks"]))


def kernel(**inputs):
    in_maps, lay = _host_prep(inputs)
    key = _sig(lay)
    if _CACHE.get("key") != key:
        _CACHE["nc"] = _build(lay)
        _CACHE["key"] = key
    res = run_bass_kernel_spmd(_CACHE["nc"], in_maps,
                               core_ids=list(range(NCORES)))
    return finalize(res, lay)


# revision 59
# speedup vs baseline: 1.0202x; 1.0202x over previous
"""ALBEF concept-text contrastive loss on 8 TRN2 NeuronCores (v5).

Design (per core r):
  * Text columns PACKED: each batch row j keeps only its valid words,
    padded to lam_j = 8*ceil(nw_j/8) (pad = copies of j's first valid
    word, mask weight 0). Bucket counts promoted to multiples of 8 so
    every core's 32-row text block has the IDENTICAL run structure
    (SPMD-safe); text rows are assigned to cores by bucket, decoupled
    from the concept sharding. TPW ~ 850 packed cols vs 1280 dense.
  * S computed as fp8 DoubleRow matmuls (K=256 in one pass), chunks of
    <=512 cols; colmax (term_col) via DVE segmented reduce per
    equal-lam run; term_row max_q via sharp LSE (ACT exp, gpsimd
    accumulate-DMA pair sums, DVE fold, one eq matmul per chunk).
  * OWN-BLOCK FIRST: core computes S for its own text block from local
    wfl8 into staging columns (g=8) while the runtime CC-init barrier
    (~43us) and first AllGather complete; the AG duplicate of the own
    block is excluded from row sums via a per-core cvalid mask and the
    host-side combine.
  * Loss: per-core [1, BT+1] partials (column exp sums + row-lse/diag
    scalar) are summed on the HOST (the unshard step) - no AllReduce.
"""

import ml_dtypes
import numpy as np

import concourse.bass as bass
import concourse.bacc as bacc
import concourse.mybir as mybir
import concourse.tile as tile
from concourse.bass_utils import run_bass_kernel_spmd

F32 = mybir.dt.float32
BF16 = mybir.dt.bfloat16
FP8 = mybir.dt.float8e4
AX = mybir.AxisListType
ALU = mybir.AluOpType
ACTF = mybir.ActivationFunctionType
PM = mybir.MatmulPerfMode

B, Q, L, VW, TW, D = 256, 32, 40, 768, 768, 256
NCORES = 8
BL = B // NCORES            # 32 local batch rows
IQ = BL * Q                 # 1024 local (q,i) columns, q-major
KC = VW // 128              # 6 contraction chunks for projection
NM = IQ // 128              # 8 m-chunks of concept rows
BT = B + BL                 # 288 sim columns (8 AG blocks + own staging)

FSC = 16.0                  # fp8 feature scale (S scaled by FSC^2=256)
BHAT = 0.375                # exp scale on scaled S (beta_orig = 96)
SHIFT = 12.0                # exp arg = BHAT*S_tilde - SHIFT
CMAX = 512                  # max matmul N / PSUM bank cols

_CACHE = {}


def _layout(text_mask):
    """Packed-column layout from the actual mask (graph-shaping)."""
    m = text_mask.astype(np.int32).copy()
    m[:, 0] = 0
    sep = (L - 1) - np.argmax(m[:, ::-1] > 0, axis=1)
    m[np.arange(B), sep] = 0
    nw = m.sum(axis=1)
    assert nw.min() >= 1
    lam = (8 * np.ceil(nw / 8.0)).astype(np.int64)
    # promote so each bucket count is a multiple of 8 (identical
    # per-core run structure); last bucket is then automatically ok
    for lv in sorted(set(lam.tolist()))[:-1]:
        js = np.where(lam == lv)[0]
        extra = len(js) % 8
        if extra:
            lam[js[np.argsort(-nw[js])[:extra]]] += 8
    vals = sorted(set(lam.tolist()), reverse=True)
    assert all((lam == lv).sum() % 8 == 0 for lv in vals)
    # deal each bucket round-robin to cores; order within core: lam desc
    order = [[] for _ in range(NCORES)]
    for lv in vals:
        js = np.where(lam == lv)[0]
        for c in range(NCORES):
            order[c].extend(js[c::NCORES].tolist())
    runs = [(lv, int((lam == lv).sum()) // 8) for lv in vals]
    tpw = int(sum(lv * nj for lv, nj in runs))
    # chunks: cut runs into <=CMAX-wide pieces, preferring run
    # boundaries (fewer reduce segments) once a chunk is >=320 wide
    chunks = []
    cur, cw, jidx = [], 0, 0
    for lv, nj in runs:
        if cw >= 320 and cw + lv * nj > CMAX:
            chunks.append((cw, cur))
            cur, cw = [], 0
        rem = nj
        while rem:
            k = min(rem, (CMAX - cw) // lv)
            if k == 0:
                chunks.append((cw, cur))
                cur, cw = [], 0
                continue
            cur.append((cw, k, lv, jidx))
            cw += k * lv
            jidx += k
            rem -= k
    if cur:
        chunks.append((cw, cur))
    # (coff, w, parts) with parts = [(off_in_chunk, nj, lam, jbase)]
    out, off = [], 0
    for w, parts in chunks:
        out.append((off, w, parts))
        off += w
    assert off == tpw
    return {"m": m, "nw": nw, "lam": lam, "order": order, "runs": runs,
            "tpw": tpw, "chunks": out}


def _build(lay):
    nc = _build_graph(lay)
    nc.compile()
    return nc


def _build_graph(lay):
    import os
    TPW = lay["tpw"]
    CH = lay["chunks"]
    MT = (TPW + 127) // 128          # text projection m-chunks
    GW0 = CH[0][1]                   # first AG group = first chunk cols
    G0M = (GW0 + 127) // 128         # text m-chunks completing group 0
    AGR = [(0, GW0), (GW0, TPW - GW0)]

    nc = bacc.Bacc("TRN2", target_bir_lowering=False, debug=False,
                   num_devices=NCORES)

    concept_t = nc.dram_tensor("concept_t", [VW, IQ], FP8, kind="ExternalInput")
    text_t = nc.dram_tensor("text_t", [TW, TPW], FP8, kind="ExternalInput")
    wc = nc.dram_tensor("wc", [VW, D], FP8, kind="ExternalInput")
    ww = nc.dram_tensor("ww", [TW, D], FP8, kind="ExternalInput")
    brows = nc.dram_tensor("brows", [1, 2 * D], BF16, kind="ExternalInput")
    ones_row = nc.dram_tensor("ones_row", [1, 128], BF16, kind="ExternalInput")
    ident_bf = nc.dram_tensor("ident_bf", [128, 128], BF16, kind="ExternalInput")
    eqmat = nc.dram_tensor("eqmat", [128, BL], BF16, kind="ExternalInput")
    eones = nc.dram_tensor("eones", [128, BL], BF16, kind="ExternalInput")
    maskw = nc.dram_tensor("maskw", [1, 9 * TPW], BF16, kind="ExternalInput")
    dmask = nc.dram_tensor("dmask", [BL, BT], F32, kind="ExternalInput")
    cvalid = nc.dram_tensor("cvalid", [BL, BT], BF16, kind="ExternalInput")
    ones32 = nc.dram_tensor("ones32", [BL, 1], F32, kind="ExternalInput")

    pout = nc.dram_tensor("pout", [1, BT + 1], F32, kind="ExternalOutput")
    dbg = None
    if os.environ.get("KDBG"):
        dbg = nc.dram_tensor("dbg", [BL, BT], F32, kind="ExternalOutput")

    # collective buffers (outputs Shared for fast HBM-HBM paths)
    ag_in = [nc.dram_tensor(f"ag_in{g}", [2 * 128, w], FP8, kind="Internal")
             for g, (_, w) in enumerate(AGR)]
    ag_out = [nc.dram_tensor(f"ag_out{g}", [NCORES * 2 * 128, w], FP8,
                             kind="Internal", addr_space="Shared")
              for g, (_, w) in enumerate(AGR)]

    with tile.TileContext(nc) as tc:
        with (
            tc.tile_pool(name="cst", bufs=1) as cst,
            tc.tile_pool(name="feat", bufs=1) as feat,
        ):
            # ---- persistent SBUF tiles ----
            cf8 = feat.tile([128, 2, IQ], FP8, tag="cf8")
            wfl8 = feat.tile([128, 2, TPW], FP8, tag="wfl8")
            wf8g = feat.tile([128, 2, NCORES * TPW], FP8, tag="wf8g")
            cm_all = feat.tile([128, NM * BT], BF16, tag="cm_all")
            trow_sb = feat.tile([BL, BT], F32, tag="trow_sb")
            sim_sb = feat.tile([BL, BT], F32, tag="sim_sb")
            qsf = feat.tile([BL, 9 * TPW], F32, tag="qsf")

            ident_sb = cst.tile([128, 128], BF16, tag="ident_sb")
            eq_sb = cst.tile([128, BL], BF16, tag="eq_sb")
            eones_sb = cst.tile([128, BL], BF16, tag="eones_sb")
            maskw_sb = cst.tile([BL, 9 * TPW], BF16, tag="maskw_sb")
            dmask_sb = cst.tile([BL, BT], F32, tag="dmask_sb")
            cvalid_sb = cst.tile([BL, BT], BF16, tag="cvalid_sb")
            ones32_sb = cst.tile([BL, 1], F32, tag="ones32_sb")
            ones32b_sb = cst.tile([BL, 1], BF16, tag="ones32b_sb")
            onesr_sb = cst.tile([1, 128], BF16, tag="onesr_sb")
            brow_sb = cst.tile([1, 2 * D], BF16, tag="brow_sb")
            shift_sb = cst.tile([128, 1], F32, tag="shift_sb")

            nc.sync.dma_start(ident_sb[:], ident_bf[:])
            nc.sync.dma_start(eq_sb[:], eqmat[:])
            nc.sync.dma_start(eones_sb[:], eones[:])
            nc.sync.dma_start(dmask_sb[:], dmask[:])
            nc.sync.dma_start(cvalid_sb[:], cvalid[:])
            nc.sync.dma_start(ones32_sb[:], ones32[:])
            nc.vector.memset(ones32b_sb[:], 1.0)
            nc.vector.memset(shift_sb[:], -SHIFT)
            nc.sync.dma_start(onesr_sb[:], ones_row[:])
            nc.sync.dma_start(brow_sb[:], brows[:])
            # load mask row once, replicate to 32 partitions by doubling
            nc.sync.dma_start(maskw_sb[0:1, :], maskw[:])
            for k in (1, 2, 4, 8, 16):
                nc.sync.dma_start(maskw_sb[k:2 * k, :], maskw_sb[0:k, :])
            # preload Square/Sqrt/Exp/Ln tables while DMAs run
            warm = cst.tile([1, 4], F32, tag="warm")
            nc.scalar.activation(warm[0:1, 3:4], ones32_sb[0:1, :], ACTF.Square)
            nc.scalar.activation(warm[0:1, 0:1], ones32_sb[0:1, :], ACTF.Sqrt)
            nc.scalar.activation(warm[0:1, 1:2], ones32_sb[0:1, :], ACTF.Exp)
            nc.scalar.activation(warm[0:1, 2:3], ones32_sb[0:1, :], ACTF.Ln)

            def issue_ag(g):
                off, w = AGR[g]
                nc.sync.dma_start(
                    ag_in[g][:].rearrange("(k p) w -> p k w", p=128),
                    wfl8[:, :, off:off + w])
                nc.gpsimd.collective_compute(
                    "AllGather", ALU.bypass,
                    ins=[ag_in[g][:]], outs=[ag_out[g][:]],
                    replica_groups=[list(range(NCORES))])
                for rr in range(NCORES):
                    for k in range(2):
                        nc.sync.dma_start(
                            wf8g[:, k, rr * TPW + off:rr * TPW + off + w],
                            ag_out[g][:].rearrange(
                                "(rr k p) w -> p k rr w",
                                k=2, p=128)[:, k, rr, :])

            # ---- stage 1: projections + l2norm -> fp8 transposed feats ----
            with (
                tc.tile_pool(name="pin", bufs=1) as pin,
                tc.tile_pool(name="ps2", bufs=3, space="PSUM") as ps2,
                tc.tile_pool(name="pst", bufs=4, space="PSUM") as pst,
                tc.tile_pool(name="wk2", bufs=3) as wk2,
            ):
                TPWP = ((TPW + 127) // 128) * 128
                tin = pin.tile([128, KC * TPWP], FP8, tag="tin")
                cin = pin.tile([128, KC * IQ], FP8, tag="cin")
                wcs = pin.tile([128, KC * D], FP8, tag="wcs")
                wws = pin.tile([128, KC * D], FP8, tag="wws")
                nc.sync.dma_start(
                    tin[:].rearrange("p (k j) -> p k j", j=TPWP)[:, :, 0:TPW],
                    text_t[:].rearrange("(k p) j -> p k j", p=128))
                nc.sync.dma_start(
                    wws[:].rearrange("p (k d) -> p k d", d=D),
                    ww[:].rearrange("(k p) d -> p k d", p=128))
                nc.sync.dma_start(
                    cin[:].rearrange("p (k j) -> p k j", j=IQ),
                    concept_t[:].rearrange("(k p) j -> p k j", p=128))
                nc.sync.dma_start(
                    wcs[:].rearrange("p (k d) -> p k d", d=D),
                    wc[:].rearrange("(k p) d -> p k d", p=128))

                def project(src, width, vwidth, w_sb, brow_ix, dst8, ms,
                            after_m=None):
                    srcv = src[:].rearrange("p (k j) -> p k j", j=vwidth)
                    wv = w_sb[:].rearrange("p (k d) -> p k d", d=D)
                    for m in ms:
                        mw = min(128, width - m * 128)
                        pp = ps2.tile([128, D], F32, tag="pp")
                        if mw >= 32:
                            for k in range(KC // 2):
                                nc.tensor.matmul(
                                    pp[0:mw, :],
                                    lhsT=srcv[:, 2 * k:2 * k + 2,
                                              m * 128:m * 128 + mw],
                                    rhs=wv[:, 2 * k:2 * k + 2, :],
                                    start=(k == 0), stop=False,
                                    perf_mode=PM.DoubleRow)
                        else:
                            for k in range(KC):
                                nc.tensor.matmul(
                                    pp[0:mw, :],
                                    lhsT=srcv[:, k:k + 1,
                                              m * 128:m * 128 + mw],
                                    rhs=wv[:, k:k + 1, :],
                                    start=(k == 0), stop=False)
                        nc.tensor.matmul(
                            pp[0:mw, :], lhsT=onesr_sb[0:1, 0:mw],
                            rhs=brow_sb[0:1, brow_ix * D:(brow_ix + 1) * D],
                            start=False, stop=True, skip_group_check=True)
                        sq = wk2.tile([128, D], BF16, tag="sq")
                        ss = wk2.tile([128, 1], F32, tag="ss")
                        nc.scalar.activation(sq[0:mw, :], pp[0:mw, :],
                                             ACTF.Square, accum_out=ss[0:mw, :])
                        rcp = wk2.tile([128, 1], F32, tag="rcp")
                        nc.vector.reciprocal(rcp[0:mw, :], ss[0:mw, :])
                        rn = wk2.tile([128, 1], F32, tag="rn")
                        # rn = FSC / sqrt(ss)
                        nc.scalar.activation(rn[0:mw, :], rcp[0:mw, :],
                                             ACTF.Sqrt, scale=FSC * FSC)
                        nrm = wk2.tile([128, D], BF16, tag="nrm")
                        nc.vector.tensor_scalar_mul(nrm[0:mw, :], pp[0:mw, :],
                                                    rn[0:mw, :])
                        for kk in range(2):
                            ptr = pst.tile([128, 128], BF16, tag="ptr")
                            nc.tensor.transpose(
                                ptr[:, 0:mw],
                                nrm[0:mw, kk * 128:(kk + 1) * 128],
                                ident_sb[0:mw, 0:mw])
                            nc.vector.tensor_scalar_mul(
                                dst8[:, kk, m * 128:m * 128 + mw],
                                ptr[:, 0:mw], 1.0)
                        if after_m is not None and m in after_m:
                            after_m[m]()

                project(tin, TPW, TPWP, wws, 1, wfl8, ms=range(0, G0M),
                        after_m={G0M - 1: lambda: issue_ag(0)})
                project(cin, IQ, IQ, wcs, 0, cf8, ms=range(NM))
                project(tin, TPW, TPWP, wws, 1, wfl8, ms=range(G0M, MT),
                        after_m={MT - 1: lambda: issue_ag(1)})

            # ---- stage 2: main pass (own block first, then AG blocks) ----
            with (
                tc.tile_pool(name="ptc", bufs=1, space="PSUM") as ptc,
                tc.tile_pool(name="psa", bufs=3, space="PSUM") as psa,
                tc.tile_pool(name="psq", bufs=1, space="PSUM") as psq,
                tc.tile_pool(name="wke", bufs=8) as wke,
                tc.tile_pool(name="wkl", bufs=3) as wkl,
            ):
                term_col = ptc.tile([BL, BT], F32, tag="term_col")

                def do_chunk(ci, g, src, scol):
                    # g: 0..7 = AG rank block, 8 = own staging block
                    coff, w, parts = CH[ci]
                    qcol = g * TPW + coff
                    exs = []
                    for pr in range(4):
                        pa = psa.tile([128, 1024], F32, tag="pa")
                        for h in range(2):
                            m = 2 * pr + h
                            nc.tensor.matmul(
                                pa[:, h * 512:h * 512 + w],
                                lhsT=cf8[:, :, m * 128:(m + 1) * 128],
                                rhs=src[:, :, scol:scol + w],
                                start=True, stop=True,
                                perf_mode=PM.DoubleRow)
                        # segmented max over l per equal-lam run
                        for poff, nj, lv, jb in parts:
                            nc.vector.tensor_reduce(
                                cm_all[:].rearrange("p (m j) -> p m j",
                                                    j=BT)[
                                    :, 2 * pr:2 * pr + 2,
                                    g * BL + jb:g * BL + jb + nj],
                                pa[:].rearrange("p (h x) -> p h x", x=512)[
                                    :, :, poff:poff + nj * lv].rearrange(
                                    "p h (j l) -> p h j l", l=lv),
                                axis=AX.X, op=ALU.max)
                        ex = wke.tile([128, 2, 512], BF16, tag="ex")
                        nc.scalar.activation(
                            ex[:, :, 0:w],
                            pa[:].rearrange("p (h x) -> p h x",
                                            x=512)[:, :, 0:w],
                            ACTF.Exp, scale=BHAT, bias=shift_sb[:])
                        exs.append(ex)
                    # sum the 4 exp tiles: 2 accum-DMAs + 1 DVE add,
                    # then fold the two 512-halves on DVE
                    nc.gpsimd.dma_start(exs[0][:, :, 0:w],
                                        exs[1][:, :, 0:w],
                                        accum_op=ALU.add)
                    nc.gpsimd.dma_start(exs[2][:, :, 0:w],
                                        exs[3][:, :, 0:w],
                                        accum_op=ALU.add)
                    exh0 = wke.tile([128, 512], BF16, tag="exh0")
                    nc.vector.tensor_tensor(exh0[:, 0:w], exs[0][:, 0, 0:w],
                                            exs[0][:, 1, 0:w], op=ALU.add)
                    exh2 = wke.tile([128, 512], BF16, tag="exh2")
                    nc.vector.tensor_tensor(exh2[:, 0:w], exs[2][:, 0, 0:w],
                                            exs[2][:, 1, 0:w], op=ALU.add)
                    qs = psq.tile([BL, 512], F32, tag="qs")
                    for t, eh in enumerate((exh0, exh2)):
                        nc.tensor.matmul(qs[0:BL, 0:w], lhsT=eq_sb[:],
                                         rhs=eh[:, 0:w],
                                         start=(t == 0), stop=(t == 1))
                    nc.scalar.copy(qsf[:, qcol:qcol + w], qs[0:BL, 0:w])

                def ln_own(ci):
                    # own block: Ln / mask / l-sum into staging cols g=8
                    coff, w, parts = CH[ci]
                    qcol = 8 * TPW + coff
                    lns = wkl.tile([BL, w], BF16, tag="lnso")
                    nc.scalar.activation(lns[:], qsf[:, qcol:qcol + w],
                                         ACTF.Ln)
                    lnw = wkl.tile([BL, w], BF16, tag="lnwo")
                    nc.vector.tensor_tensor(lnw[:], lns[:],
                                            maskw_sb[:, qcol:qcol + w],
                                            op=ALU.mult)
                    for poff, nj, lv, jb in parts:
                        nc.vector.tensor_reduce(
                            trow_sb[:, 8 * BL + jb:8 * BL + jb + nj],
                            lnw[:, poff:poff + nj * lv].rearrange(
                                "p (j l) -> p j l", l=lv),
                            axis=AX.X, op=ALU.add)

                def ln_batch(ci, r0=0, r1=NCORES):
                    # AG blocks: batched Ln / mask-mult over rank range
                    # [r0, r1) (3D APs, stride TPW)
                    coff, w, parts = CH[ci]
                    nr = r1 - r0
                    qv = qsf[:, 0:NCORES * TPW].rearrange(
                        "p (rr x) -> p rr x",
                        x=TPW)[:, r0:r1, coff:coff + w]
                    lns = wkl.tile([BL, NCORES, 512], BF16, tag="lns")
                    nc.scalar.activation(lns[0:BL, 0:nr, 0:w], qv, ACTF.Ln)
                    mv = maskw_sb[:, 0:NCORES * TPW].rearrange(
                        "p (rr x) -> p rr x",
                        x=TPW)[:, r0:r1, coff:coff + w]
                    lnw = wkl.tile([BL, NCORES, 512], BF16, tag="lnw")
                    nc.vector.tensor_tensor(lnw[0:BL, 0:nr, 0:w],
                                            lns[0:BL, 0:nr, 0:w], mv,
                                            op=ALU.mult)
                    for poff, nj, lv, jb in parts:
                        nc.vector.tensor_reduce(
                            trow_sb[:].rearrange("p (g j) -> p g j",
                                                 j=BL)[:, r0:r1,
                                                       jb:jb + nj],
                            lnw[0:BL, 0:nr,
                                poff:poff + nj * lv].rearrange(
                                "p rr (j l) -> p rr j l", l=lv),
                            axis=AX.X, op=ALU.add)

                # own block from local features (fills the CC-init stall)
                for ci in range(len(CH)):
                    do_chunk(ci, 8, wfl8, CH[ci][0])
                    ln_own(ci)
                def term_col_ci(ci):
                    # fold this ci's colmax cols over q now (j cols of a
                    # chunk are contiguous within every block)
                    j0 = CH[ci][2][0][3]
                    j1 = CH[ci][2][-1][3] + CH[ci][2][-1][1]
                    for m in range(NM):
                        nc.tensor.matmul(
                            term_col[:].rearrange("p (g j) -> p g j",
                                                  j=BL)[:, :, j0:j1],
                            lhsT=eones_sb[:],
                            rhs=cm_all[:, m * BT:(m + 1) * BT].rearrange(
                                "p (g j) -> p g j", j=BL)[:, :, j0:j1],
                            start=(m == 0), stop=(m == NM - 1))

                # gathered blocks; defer each ci's Ln/term_col epilogue
                # into the next ci's compute to avoid boundary stalls
                NCH = len(CH)
                for ci in range(NCH):
                    for rr in range(NCORES):
                        do_chunk(ci, rr, wf8g, rr * TPW + CH[ci][0])
                        if ci > 0 and rr == 2:
                            ln_batch(ci - 1, 0, 4)
                        if ci > 0 and rr == 4:
                            ln_batch(ci - 1, 4, NCORES)
                        if ci > 0 and rr == 6:
                            term_col_ci(ci - 1)
                        # last ci: drain ranks 0-5 early so only 6-7
                        # remain on the serial tail
                        if ci == NCH - 1 and rr == 6:
                            ln_batch(ci, 0, 6)
                ln_batch(NCH - 1, 6, NCORES)
                term_col_ci(NCH - 1)

                # ---- stage 3: sim ----
                nc.vector.tensor_tensor(sim_sb[:], term_col[:], trow_sb[:],
                                        op=ALU.add)
                if dbg is not None:
                    nc.sync.dma_start(dbg[:], sim_sb[:])

            # ---- loss partials; cross-core combine happens on host ----
            with (
                tc.tile_pool(name="ps7", bufs=1, space="PSUM") as ps7,
                tc.tile_pool(name="wk7", bufs=1) as wk7,
            ):
                # sim is O(+-30): exp needs no max shift in f32/bf16
                escr = wk7.tile([BL, BT], BF16, tag="escr")
                nc.scalar.activation(escr[:], sim_sb[:], ACTF.Exp)
                escv = wk7.tile([BL, BT], BF16, tag="escv")
                sume = wk7.tile([BL, 1], F32, tag="sume")
                nc.vector.scalar_tensor_tensor(
                    escv[:], escr[:], 1.0, cvalid_sb[:],
                    op0=ALU.mult, op1=ALU.mult, accum_out=sume[:])
                lg = wk7.tile([BL, 1], F32, tag="lg")
                nc.scalar.activation(lg[:], sume[:], ACTF.Ln)
                dscr = wk7.tile([BL, BT], F32, tag="dscr")
                dg = wk7.tile([BL, 1], F32, tag="dg")
                nc.vector.scalar_tensor_tensor(
                    dscr[:], sim_sb[:], 1.0, dmask_sb[:],
                    op0=ALU.mult, op1=ALU.mult, accum_out=dg[:])
                # v = 2*dg - lse_row  [BL,1]
                v1 = wk7.tile([BL, 1], F32, tag="v1")
                nc.vector.scalar_tensor_tensor(
                    v1[:], dg[:], 2.0, lg[:],
                    op0=ALU.mult, op1=ALU.subtract)
                csum = ps7.tile([1, BT], F32, tag="csum")
                nc.tensor.matmul(csum[:], lhsT=ones32b_sb[:],
                                 rhs=escv[:], start=True, stop=True)
                ssum = ps7.tile([1, 1], F32, tag="ssum")
                nc.tensor.matmul(ssum[:], lhsT=ones32_sb[:], rhs=v1[:],
                                 start=True, stop=True)
                arv = wk7.tile([1, BT + 1], F32, tag="arv")
                nc.scalar.copy(arv[0:1, 0:BT], csum[0:1, :])
                nc.scalar.copy(arv[0:1, BT:BT + 1], ssum[0:1, :])
                nc.sync.dma_start(pout[:], arv[:])

    return nc


def _host_prep(inputs):
    concept_feat = np.ascontiguousarray(np.asarray(inputs["concept_feat"],
                                                   dtype=np.float32))
    text_embeds = np.asarray(inputs["text_embeds"], dtype=np.float32)
    text_mask = np.asarray(inputs["text_mask"]).astype(np.int32)
    Wc = np.ascontiguousarray(np.asarray(inputs["Wc"], dtype=np.float32))
    bc = np.asarray(inputs["bc"], dtype=np.float32)
    Ww = np.ascontiguousarray(np.asarray(inputs["Ww"], dtype=np.float32))
    bw = np.asarray(inputs["bw"], dtype=np.float32)
    temp = float(np.asarray(inputs["temp_cpt"]))

    lay = _layout(text_mask)
    m, nw, lam = lay["m"], lay["nw"], lay["lam"]
    order, TPW = lay["order"], lay["tpw"]

    # Eq: partition p=(qo,i) -> i one-hot; eones adds 1/(Q*temp*FSC^2)
    eq = np.zeros((128, BL), dtype=np.float32)
    eo = np.zeros((128, BL), dtype=np.float32)
    for p in range(128):
        eq[p, p % BL] = 1.0
        eo[p, p % BL] = 1.0 / (Q * temp * FSC * FSC)

    # packed per-core text blocks + global mask-weight row
    wscale = 1.0 / (temp * FSC * FSC * BHAT)
    blocks, wrow_blocks, pos_of = [], [], {}
    for r in range(NCORES):
        cols = np.empty((TPW, TW), dtype=np.float32)
        wrow = np.zeros((TPW,), dtype=np.float32)
        off = 0
        for pos, j in enumerate(order[r]):
            lv, nv = int(lam[j]), int(nw[j])
            pos_of[j] = (r, pos)
            valid = np.where(m[j] > 0)[0]
            cols[off:off + nv] = text_embeds[j, valid]
            cols[off + nv:off + lv] = text_embeds[j, valid[0]]
            wrow[off:off + nv] = wscale / nv
            off += lv
        assert off == TPW
        blocks.append(cols)
        wrow_blocks.append(wrow)
    maskw_g = np.concatenate(wrow_blocks)

    ident = np.eye(128, dtype=np.float32)
    ones_row = np.ones((1, 128), dtype=np.float32)
    ones32 = np.ones((BL, 1), dtype=np.float32)
    brows = np.concatenate([bc, bw])[None, :]

    # weights/bias fp8, x64 into e4m3 normal range; l2norm removes the
    # common scale so no unscaling is needed downstream
    WS = 64.0
    shared = {
        "wc": (Wc * WS).astype(ml_dtypes.float8_e4m3),
        "ww": (Ww * WS).astype(ml_dtypes.float8_e4m3),
        "brows": (brows * WS).astype(ml_dtypes.bfloat16),
        "ones_row": ones_row.astype(ml_dtypes.bfloat16),
        "ident_bf": ident.astype(ml_dtypes.bfloat16),
        "eqmat": eq.astype(ml_dtypes.bfloat16),
        "eones": eo.astype(ml_dtypes.bfloat16),
        "ones32": ones32,
    }
    in_maps = []
    for r in range(NCORES):
        im = dict(shared)
        im["concept_t"] = np.ascontiguousarray(
            concept_feat[r * BL:(r + 1) * BL].transpose(1, 0, 2)
            .reshape(IQ, VW).T).astype(ml_dtypes.float8_e4m3)
        im["text_t"] = np.ascontiguousarray(
            blocks[r].T).astype(ml_dtypes.float8_e4m3)
        im["maskw"] = np.concatenate(
            [maskw_g, wrow_blocks[r]])[None, :].astype(ml_dtypes.bfloat16)
        dmask_np = np.zeros((BL, BT), dtype=np.float32)
        cval = np.ones((BL, BT), dtype=np.float32)
        cval[:, r * BL:(r + 1) * BL] = 0.0
        for i in range(BL):
            tr, pos = pos_of[r * BL + i]
            col = 8 * BL + pos if tr == r else tr * BL + pos
            dmask_np[i, col] = 1.0
        im["dmask"] = dmask_np
        im["cvalid"] = cval.astype(ml_dtypes.bfloat16)
        in_maps.append(im)
    return in_maps, lay


def finalize(res, lay):
    # host-side unshard: sum the per-core [1, BT+1] partials; own-block
    # staging cols (g=8) fill the cvalid-zeroed own-rank AG slots
    parts = np.stack([np.asarray(res.results[r]["pout"][0],
                                 dtype=np.float64) for r in range(NCORES)])
    tot = parts[:, 0:B].sum(axis=0)
    for r in range(NCORES):
        tot[r * BL:(r + 1) * BL] += parts[r, B:BT]
    v = parts[:, BT].sum()
    fin = np.log(tot).sum() - v
    return np.float32(fin / (2 * B))


def _sig(lay):
    return (lay["tpw"], tuple(lay["runs"]),
            tuple((c, w, tuple(p)) for c, w, p in lay["chunks"]))


def kernel(**inputs):
    in_maps, lay = _host_prep(inputs)
    key = _sig(lay)
    if _CACHE.get("key") != key:
        _CACHE["nc"] = _build(lay)
        _CACHE["key"] = key
    res = run_bass_kernel_spmd(_CACHE["nc"], in_maps,
                               core_ids=list(range(NCORES)))
    return finalize(res, lay)
